# revision 35
# baseline (speedup 1.0000x reference)
"""GPT block (LN -> causal MHA -> LN -> MLP) on 8 TRN2 NeuronCores.

Sharding: each core owns one (batch, query-chunk-pair). B=4 batches x 2
chunk-pairs = 8 cores. Chunk pairs are zig-zag ({0,3} / {1,2}) over four
512-row chunks of T=2048 so attention work balances. Each core recomputes
K/V for the full sequence locally (no collectives), runs flash-style
attention for its 1024 query rows, then the MLP for the same rows.

All activations live feature-on-partition ("transposed"); the host
pre-transposes x and assembles the output. Per-core causality is handled
with a block permutation of the sequence; full-block allow/deny is driven
by per-core exp scale/bias inputs (exp(0*s - 1e9) = 0 kills forbidden
blocks) and the partial diagonal blocks by static 0/1 bf16 masks applied
post-exp.

Schedule (v3): one software-pipelined stream so the PE never idles on the
Act engine's exp:
  LN1 (chunk-pipelined) -> QKV (minus K/V tiles 12-15) ->
  [attention slot A || deferred K/V GEMMs] ->
  [attention slot B || attnproj-A + LN2-A] ->
  [FC-A, proj-A || attnproj-B, LN2-B, FC-B] -> proj-B.
Exp is batched over two 512-col score tiles in adjacent PSUM banks; LN
gamma/beta ride the next GEMM's weights (host-folded); LN1 stats matmuls
run in float32r (no bf16 cast); softmax denominators use
reciprocal_approx_fast + Pool-engine partition broadcast.
"""

import numpy as np
import ml_dtypes

BF = ml_dtypes.bfloat16

E = 1024          # embedding
T = 2048          # sequence
B = 4             # batch
NH = 16           # heads
D = 64            # head dim
HID = 4096        # mlp hidden
KT = E // 128     # k-tiles over embedding (8)
CH = 512          # chunk rows
NEG = -1.0e9
EPS = 1e-5

_CACHE = {}


def _build_program():
    import concourse.bass as bass
    import concourse.tile as tile
    from concourse import bacc, mybir

    f32 = mybir.dt.float32
    f32r = mybir.dt.float32r
    bf16 = mybir.dt.bfloat16
    AF = mybir.ActivationFunctionType
    ALU = mybir.AluOpType

    nc = bacc.Bacc()

    xT_d = nc.declare_dram_parameter("xT", [E, T], f32, isOutput=False)
    w_attn_d = nc.declare_dram_parameter("w_attn", [E, 3 * E], bf16, isOutput=False)
    b_qk_d = nc.declare_dram_parameter("b_qk", [2 * E, 1], f32, isOutput=False)
    b_v_d = nc.declare_dram_parameter("b_v", [1, E], f32, isOutput=False)
    w_ap_d = nc.declare_dram_parameter("w_ap", [E, E], bf16, isOutput=False)
    b_ap_d = nc.declare_dram_parameter("b_ap", [E, 1], f32, isOutput=False)
    w_fc_d = nc.declare_dram_parameter("w_fc", [E, HID], bf16, isOutput=False)
    b_fc_d = nc.declare_dram_parameter("b_fc", [HID, 1], f32, isOutput=False)
    w_pr_d = nc.declare_dram_parameter("w_proj", [HID, E], bf16, isOutput=False)
    b_pr_d = nc.declare_dram_parameter("b_proj", [E, 1], f32, isOutput=False)
    pm_d = nc.declare_dram_parameter("pmask", [2, 128, 2 * CH], bf16, isOutput=False)
    sA_s_d = nc.declare_dram_parameter("sA_scale", [128, 1], f32, isOutput=False)
    sA_b_d = nc.declare_dram_parameter("sA_bias", [128, 1], f32, isOutput=False)
    sB_s_d = nc.declare_dram_parameter("sB_scale", [128, 3], f32, isOutput=False)
    sB_b_d = nc.declare_dram_parameter("sB_bias", [128, 3], f32, isOutput=False)
    out_d = nc.declare_dram_parameter("outT", [E, 2 * CH], f32, isOutput=True)

    wT = w_attn_d.rearrange("(k p) n -> p k n", p=128)
    wfcT = w_fc_d.rearrange("(k p) n -> p k n", p=128)
    wapT = w_ap_d.rearrange("(k p) n -> p k n", p=128)
    wprT = w_pr_d.rearrange("(k p) n -> p k n", p=128)
    xTr = xT_d.rearrange("(k p) n -> p k n", p=128)

    with tile.TileContext(nc) as tc:
        from contextlib import ExitStack

        stack = ExitStack()
        with stack:
            # ---- long-lived left-side pools (LIFO nesting) ----
            const = stack.enter_context(tc.tile_pool(name="const", bufs=1))
            aBp = stack.enter_context(tc.tile_pool(name="aBp", bufs=1))
            h2p = stack.enter_context(tc.tile_pool(name="h2p", bufs=1))
            # ---- right-side ----
            x2p = stack.enter_context(tc.tile_pool(name="x2p", bufs=1, side="right"))

            ones_col_bf = const.tile([128, 1], bf16)
            nc.vector.memset(ones_col_bf[:], 1.0)
            ones_col_f = const.tile([128, 1], f32)
            nc.vector.memset(ones_col_f[:], 1.0)
            eps_t = const.tile([1, 1], f32)
            nc.vector.memset(eps_t[:], EPS)

            pmask = const.tile([128, 2, 2 * CH], bf16)
            nc.sync.dma_start(pmask[:], pm_d.rearrange("v p n -> p v n"))
            sA_s = const.tile([128, 1], f32)
            nc.sync.dma_start(sA_s[:], sA_s_d[:])
            sA_b = const.tile([128, 1], f32)
            nc.sync.dma_start(sA_b[:], sA_b_d[:])
            sB_s = const.tile([128, 3], f32)
            nc.sync.dma_start(sB_s[:], sB_s_d[:])
            sB_b = const.tile([128, 3], f32)
            nc.sync.dma_start(sB_b[:], sB_b_d[:])

            b_qk = const.tile([128, 16, 1], f32)
            nc.sync.dma_start(b_qk[:], b_qk_d.rearrange("(k p) o -> p k o", p=128))
            b_v = const.tile([1, E], f32)
            nc.sync.dma_start(b_v[:], b_v_d[:])
            b_ap = const.tile([128, KT, 1], f32)
            nc.sync.dma_start(b_ap[:], b_ap_d.rearrange("(k p) o -> p k o", p=128))
            b_fc = const.tile([128, 32, 1], f32)
            nc.sync.dma_start(b_fc[:], b_fc_d.rearrange("(k p) o -> p k o", p=128))
            b_pr = const.tile([128, KT, 1], f32)
            nc.sync.dma_start(b_pr[:], b_pr_d.rearrange("(k p) o -> p k o", p=128))

            attnTB = aBp.tile([128, KT, CH], bf16)
            h2T = h2p.tile([128, KT, 2 * CH], bf16)
            x2T = x2p.tile([128, KT, 2 * CH], bf16)

            # window-scoped pools (opened/closed in LIFO order)
            aAp = stack.enter_context(tc.tile_pool(name="aAp", bufs=1))
            attnTA = aAp.tile([128, KT, CH], bf16)
            attnTs = [attnTA, attnTB]

            atsb_ctx = ExitStack()
            atsb = atsb_ctx.enter_context(tc.tile_pool(name="atsb", bufs=4))

            # ---------------- Phase 1: LN1 (chunk-pipelined) ----------------
            ln1_ctx = ExitStack()
            ln1 = ln1_ctx.enter_context(tc.tile_pool(name="ln1", bufs=1))
            ln1_t = ln1.tile([128, KT, T], bf16)

            with tc.tile_pool(name="ln1ps", bufs=1, space="PSUM") as lnps, \
                 tc.tile_pool(name="ln1sb", bufs=1) as lnsb:
                state = {}

                def ln1_stats(ch):
                    xt = lnsb.tile([128, KT, CH], f32, tag="xin", bufs=2,
                                   name=f"l1x{ch}")
                    nc.sync.dma_start(xt[:], xTr[:, :, ch * CH:(ch + 1) * CH])
                    mu_ps = lnps.tile([1, CH], f32, tag="stat", bufs=4,
                                      name=f"l1mu{ch}")
                    ss_ps = lnps.tile([1, CH], f32, tag="stat", bufs=4,
                                      name=f"l1ss{ch}")
                    for kt in range(KT):
                        xbf = lnsb.tile([128, CH], bf16, tag="xbf", bufs=3,
                                        name=f"l1xb{ch}_{kt}")
                        if kt % 2 == 0:
                            nc.scalar.activation(xbf[:], xt[:, kt, :], AF.Copy)
                        else:
                            nc.vector.tensor_copy(xbf[:], xt[:, kt, :])
                        sq = lnsb.tile([128, CH], bf16, tag="sq", bufs=3,
                                       name=f"l1sq{ch}_{kt}")
                        nc.scalar.square(sq[:], xbf[:])
                        nc.tensor.matmul(mu_ps[:], ones_col_bf[:], xbf[:],
                                         start=(kt == 0), stop=(kt == KT - 1))
                        nc.tensor.matmul(ss_ps[:], ones_col_bf[:], sq[:],
                                         start=(kt == 0), stop=(kt == KT - 1))
                    state[ch] = (xt, mu_ps, ss_ps)

                def ln1_norm(ch):
                    xt, mu_ps, ss_ps = state.pop(ch)
                    mu = lnsb.tile([1, CH], f32, tag="row", bufs=8)
                    nc.scalar.activation(mu[:], mu_ps[:], AF.Copy,
                                         scale=1.0 / E)
                    ms = lnsb.tile([1, CH], f32, tag="row", bufs=8)
                    nc.scalar.activation(ms[:], ss_ps[:], AF.Copy,
                                         scale=1.0 / E)
                    msq = lnsb.tile([1, CH], f32, tag="row", bufs=8)
                    nc.scalar.square(msq[:], mu[:])
                    var = lnsb.tile([1, CH], f32, tag="row", bufs=8)
                    nc.vector.tensor_sub(var[:], ms[:], msq[:])
                    sd = lnsb.tile([1, CH], f32, tag="row", bufs=8)
                    nc.scalar.activation(sd[:], var[:], AF.Sqrt,
                                         bias=eps_t[:])
                    rstd = lnsb.tile([1, CH], f32, tag="row", bufs=8)
                    nc.vector.reciprocal_approx_fast(rstd[:], sd[:])
                    nmr = lnsb.tile([1, CH], f32, tag="row", bufs=8)
                    nc.vector.scalar_tensor_tensor(
                        nmr[:], mu[:], -1.0, rstd[:], ALU.mult, ALU.mult)
                    a_bc = lnsb.tile([128, CH], f32, tag="bc", bufs=4)
                    nc.gpsimd.partition_broadcast(a_bc[:], rstd[:])
                    c_bc = lnsb.tile([128, CH], f32, tag="bc", bufs=4)
                    nc.gpsimd.partition_broadcast(c_bc[:], nmr[:])
                    for kt in range(KT):
                        t1 = lnsb.tile([128, CH], f32, tag="t1", bufs=3)
                        nc.vector.tensor_mul(t1[:], xt[:, kt, :], a_bc[:])
                        nc.vector.tensor_add(
                            ln1_t[:, kt, ch * CH:(ch + 1) * CH], t1[:], c_bc[:])

                for ch in range(5):
                    if ch < 4:
                        ln1_stats(ch)
                    if ch >= 1:
                        ln1_norm(ch - 1)

            # ---------------- Phase 2: QKV (minus deferred) ----------------
            qA_ctx = ExitStack()
            qAp = qA_ctx.enter_context(tc.tile_pool(name="qAp", bufs=1))
            qTA = qAp.tile([128, KT, CH], bf16)
            qkv_ctx = ExitStack()
            qkvp = qkv_ctx.enter_context(
                tc.tile_pool(name="qkvp", bufs=1, side="right"))
            qTB = qkvp.tile([128, KT, CH], bf16)
            qTs = [qTA, qTB]
            kT = qkvp.tile([128, KT, T], bf16)
            v_aug = qkvp.tile([128, 16, NH * 65], bf16)
            v4 = v_aug.rearrange("p m (h w) -> p m h w", h=NH)

            with tc.tile_pool(name="wqk", bufs=2) as wqk_pool, \
                 tc.tile_pool(name="qkps", bufs=4, space="PSUM") as qkps:
                for g in range(8):
                    panel = wqk_pool.tile([128, KT, 256], bf16, tag="w")
                    nc.sync.dma_start(panel[:], wT[:, :, g * 256:(g + 1) * 256])
                    for mm in range(2):
                        mt = 2 * g + mm
                        is_q = mt < 8
                        n_chunks = 2 if is_q else 4
                        dt_idx = mt if is_q else mt - 8
                        for nq in range(n_chunks):
                            ps = qkps.tile([128, CH], f32, tag="ps")
                            for kt in range(KT):
                                nc.tensor.matmul(
                                    ps[:], panel[:, kt, mm * 128:(mm + 1) * 128],
                                    ln1_t[:, kt, nq * CH:(nq + 1) * CH],
                                    start=(kt == 0), stop=(kt == KT - 1))
                            dst_ap = (qTs[nq][:, dt_idx, :] if is_q else
                                      kT[:, dt_idx, nq * CH:(nq + 1) * CH])
                            nc.scalar.activation(
                                dst_ap, ps[:],
                                AF.Identity, bias=b_qk[:, mt, 0:1])
                for g in range(2):
                    panel = wqk_pool.tile([128, KT, CH], bf16, tag="wv",
                                          bufs=1)
                    nc.sync.dma_start(
                        panel[:], wT[:, :, 2 * E + g * CH:2 * E + (g + 1) * CH])
                    bv_sb = wqk_pool.tile([128, CH], f32, tag="bvs", bufs=1)
                    nc.gpsimd.partition_broadcast(
                        bv_sb[:], b_v[:, g * CH:(g + 1) * CH])
                    bv_sb3 = bv_sb.rearrange("p (h w) -> p h w", h=8)
                    for mv in range(16):
                        ps = qkps.tile([128, CH], f32, tag="ps")
                        for kt in range(KT):
                            nc.tensor.matmul(
                                ps[:], ln1_t[:, kt, mv * 128:(mv + 1) * 128],
                                panel[:, kt, :],
                                start=(kt == 0), stop=(kt == KT - 1))
                        ps3 = ps.rearrange("p (h w) -> p h w", h=8)
                        nc.vector.tensor_add(
                            v4[:, mv, g * 8:(g + 1) * 8, 0:64], ps3[:], bv_sb3[:])
                for mv in range(16):
                    nc.vector.memset(v4[:, mv, :, 64:65], 1.0)

            # attention PSUM pool: s 3x2 banks + av 2 = 8 banks
            atps_ctx = ExitStack()
            atps = atps_ctx.enter_context(
                tc.tile_pool(name="atps", bufs=1, space="PSUM"))

            # ============ attention machinery ============
            pairs_a = [(0, 1, ("diag", 0)), (2, 3, ("diag", 1)),
                       (8, 9, ("drv", "A", 0)), (10, 11, ("drv", "A", 0))]
            pairs_b = [(4, 5, ("diag", 0)), (6, 7, ("diag", 1)),
                       (0, 1, ("drv", "B", 0)), (2, 3, ("drv", "B", 0)),
                       (8, 9, ("drv", "B", 1)), (10, 11, ("drv", "B", 1)),
                       (12, 13, ("drv", "B", 2)), (14, 15, ("drv", "B", 2))]

            avs = {}
            ess = {}

            def emit_s_exp(work, i):
                h, slot, j, npairs, (t0, t1, mk) = work[i]
                ktf = h // 2
                ro = (h % 2) * 64
                s_ps = atps.tile([128, 2 * CH], f32, tag="s", bufs=3)
                for k, t in enumerate((t0, t1)):
                    nc.tensor.matmul(
                        s_ps[:, k * CH:(k + 1) * CH],
                        kT[ro:ro + 64, ktf, t * 128:(t + 1) * 128],
                        qTs[slot][ro:ro + 64, ktf, :],
                        start=True, stop=True)
                es = atsb.tile([128, 2 * CH], bf16, tag="es", bufs=4)
                if mk[0] == "diag":
                    nc.scalar.activation(es[:], s_ps[:], AF.Exp)
                    nc.vector.tensor_mul(es[:], es[:], pmask[:, mk[1], :])
                else:
                    sc = sA_s if mk[1] == "A" else sB_s
                    bi = sA_b if mk[1] == "A" else sB_b
                    idx = mk[2]
                    nc.scalar.activation(
                        es[:], s_ps[:], AF.Exp,
                        bias=bi[:, idx:idx + 1],
                        scale=sc[:, idx:idx + 1])
                ess[(h, slot, j)] = es

            def emit_pv(work, i):
                h, slot, j, npairs, (t0, t1, mk) = work[i]
                if j == 0:
                    avs[(h, slot)] = atps.tile([65, CH], f32, tag="av",
                                               bufs=2, name=f"av{h}_{slot}")
                out_ps = avs[(h, slot)]
                es = ess.pop((h, slot, j))
                for k, t in enumerate((t0, t1)):
                    nc.tensor.matmul(
                        out_ps[:], v_aug[:, t, h * 65:(h + 1) * 65],
                        es[:, k * CH:(k + 1) * CH],
                        start=(j == 0 and k == 0),
                        stop=(j == npairs - 1 and k == 1))
                if j == npairs - 1:
                    ktf = h // 2
                    ro = (h % 2) * 64
                    den = atsb.tile([1, CH], f32, tag="dn", bufs=2)
                    nc.vector.tensor_copy(den[:], out_ps[64:65, :])
                    rec = atsb.tile([1, CH], f32, tag="rc", bufs=2)
                    nc.vector.reciprocal_approx_fast(rec[:], den[:])
                    bc64 = atsb.tile([64, CH], f32, tag="bcr", bufs=2)
                    nc.gpsimd.partition_broadcast(bc64[:], rec[:])
                    nc.vector.tensor_mul(
                        attnTs[slot][ro:ro + 64, ktf, :], out_ps[0:64, :],
                        bc64[:])

            def attn_stream(slot):
                seq = pairs_a if slot == 0 else pairs_b
                work = []
                for h in range(NH):
                    for j, p in enumerate(seq):
                        work.append((h, slot, j, len(seq), p))
                LEAD = 2
                for i in range(len(work) + LEAD):
                    if i < len(work):
                        emit_s_exp(work, i)
                    if i >= LEAD:
                        emit_pv(work, i - LEAD)
                    yield

            wap_box = {}

            def ap_ln2_stream(slot, mlsb):
                """attnproj + residual -> LN2 for one slot."""
                nq = slot
                if "w" not in wap_box:
                    wap_box["w"] = mlsb.tile([128, KT, E], bf16, tag="wapf",
                                             bufs=1, name="wapfull")
                    nc.sync.dma_start(wap_box["w"][:], wapT[:])
                w_ap = wap_box["w"]
                for m in range(KT):
                    ps = mlps.tile([128, CH], f32, tag="ps", bufs=2,
                                   name=f"ap{slot}_{m}")
                    for kt in range(KT):
                        nc.tensor.matmul(
                            ps[:], w_ap[:, kt, m * 128:(m + 1) * 128],
                            attnTs[slot][:, kt, :],
                            start=(kt == 0), stop=(kt == KT - 1))
                    xq = mlsb.tile([128, CH], f32, tag="xq", bufs=2,
                                   name=f"xq{slot}_{m}")
                    nc.sync.dma_start(
                        xq[:], xT_d[m * 128:(m + 1) * 128,
                                    nq * CH:(nq + 1) * CH])
                    nc.vector.scalar_tensor_tensor(
                        x2T[:, m, nq * CH:(nq + 1) * CH], ps[:],
                        b_ap[:, m, 0:1], xq[:], ALU.add, ALU.add)
                    yield
                # LN2 (stats borrow partitions 0/32 of a [128, CH] psum tile;
                # x2 is bf16 so the stats matmuls run in bf16 directly)
                stat_t = mlps.tile([128, CH], f32, tag="ps", bufs=2,
                                   name=f"l2s{slot}")
                mu_ps = stat_t[0:1, :]
                ss_ps = stat_t[32:33, :]
                src = x2T[:, :, nq * CH:(nq + 1) * CH]
                for kt in range(KT):
                    sq = mlsb.tile([128, CH], bf16, tag="sq2", bufs=2,
                                   name=f"l2sq{slot}_{kt}")
                    nc.scalar.square(sq[:], src[:, kt, :])
                    nc.tensor.matmul(mu_ps, ones_col_bf[:], src[:, kt, :],
                                     start=(kt == 0), stop=(kt == KT - 1))
                    nc.tensor.matmul(ss_ps, ones_col_bf[:], sq[:],
                                     start=(kt == 0), stop=(kt == KT - 1))
                    if kt % 2 == 1:
                        yield
                mu = mlsb.tile([1, CH], f32, tag="row2", bufs=7)
                nc.scalar.activation(mu[:], mu_ps, AF.Copy, scale=1.0 / E)
                ms = mlsb.tile([1, CH], f32, tag="row2", bufs=7)
                nc.scalar.activation(ms[:], ss_ps, AF.Copy, scale=1.0 / E)
                msq = mlsb.tile([1, CH], f32, tag="row2", bufs=7)
                nc.scalar.square(msq[:], mu[:])
                var = mlsb.tile([1, CH], f32, tag="row2", bufs=7)
                nc.vector.tensor_sub(var[:], ms[:], msq[:])
                sd = mlsb.tile([1, CH], f32, tag="row2", bufs=7)
                nc.scalar.activation(sd[:], var[:], AF.Sqrt, bias=eps_t[:])
                rstd = mlsb.tile([1, CH], f32, tag="row2", bufs=7)
                nc.vector.reciprocal_approx_fast(rstd[:], sd[:])
                nmr = mlsb.tile([1, CH], f32, tag="row2", bufs=7)
                nc.vector.scalar_tensor_tensor(
                    nmr[:], mu[:], -1.0, rstd[:], ALU.mult, ALU.mult)
                a_bc = mlsb.tile([128, CH], f32, tag="bc2", bufs=2)
                nc.gpsimd.partition_broadcast(a_bc[:], rstd[:])
                c_bc = mlsb.tile([128, CH], f32, tag="bc2", bufs=2)
                nc.gpsimd.partition_broadcast(c_bc[:], nmr[:])
                for kt in range(KT):
                    t1 = mlsb.tile([128, CH], f32, tag="t12", bufs=2)
                    nc.vector.tensor_mul(t1[:], src[:, kt, :], a_bc[:])
                    nc.vector.tensor_add(
                        h2T[:, kt, nq * CH:(nq + 1) * CH], t1[:], c_bc[:])
                    if kt % 2 == 1:
                        yield

            def fc_stream(slot, g_t, mlsb):
                nq = slot
                for mg in range(8):
                    panel = mlsb.tile([128, KT, CH], bf16, tag="wfc", bufs=1,
                                      name=f"fc{slot}_{mg}")
                    nc.sync.dma_start(
                        panel[:], wfcT[:, :, mg * CH:(mg + 1) * CH])
                    for mm in range(4):
                        ps = mlps.tile([128, CH], f32, tag="ps", bufs=2,
                                       name=f"fc{slot}_{mg}_{mm}")
                        for kt in range(KT):
                            nc.tensor.matmul(
                                ps[:], panel[:, kt, mm * 128:(mm + 1) * 128],
                                h2T[:, kt, nq * CH:(nq + 1) * CH],
                                start=(kt == 0), stop=(kt == KT - 1))
                        mt = mg * 4 + mm
                        nc.scalar.activation(
                            g_t[:, mt, nq * CH:(nq + 1) * CH], ps[:],
                            AF.Gelu, bias=b_fc[:, mt, 0:1])
                        yield

            def proj_stream(slot, g_t, prps, prsb):
                nq = slot
                for half in range(2):
                    ms = range(half * 4, (half + 1) * 4)
                    pss = [prps.tile([128, CH], f32, tag="ps", bufs=4,
                                     name=f"pr{slot}_{half}_{m}")
                           for m in ms]
                    for kg in range(8):
                        panel = prsb.tile([128, 4, E], bf16, tag="w", bufs=2,
                                          name=f"prw{slot}_{half}_{kg}")
                        nc.sync.dma_start(
                            panel[:], wprT[:, 4 * kg:4 * kg + 4, :])
                        for kk in range(4):
                            kt = kg * 4 + kk
                            for mi, m in enumerate(ms):
                                nc.tensor.matmul(
                                    pss[mi][:],
                                    panel[:, kk, m * 128:(m + 1) * 128],
                                    g_t[:, kt, nq * CH:(nq + 1) * CH],
                                    start=(kt == 0), stop=(kt == 31),
                                    skip_group_check=True)
                        yield
                    for mi, m in enumerate(ms):
                        ot = prsb.tile([128, CH], f32, tag="ot", bufs=3,
                                       name=f"ot{slot}_{half}_{m}")
                        nc.vector.scalar_tensor_tensor(
                            ot[:], pss[mi][:], b_pr[:, m, 0:1],
                            x2T[:, m, nq * CH:(nq + 1) * CH],
                            ALU.add, ALU.add)
                        nc.sync.dma_start(
                            out_d[m * 128:(m + 1) * 128,
                                  nq * CH:(nq + 1) * CH],
                            ot[:])
                        yield

            def chain(*gens):
                for g in gens:
                    for _ in g:
                        yield

            def interleave(main, filler, ratio):
                budget = 0.0
                for _ in main:
                    budget += ratio
                    while budget >= 1.0 and filler is not None:
                        budget -= 1.0
                        try:
                            next(filler)
                        except StopIteration:
                            filler = None
                if filler is not None:
                    for _ in filler:
                        pass

            # ---- window 1: attention A ----
            for _ in attn_stream(0):
                pass
            qA_ctx.close()
            ln1_ctx.close()

            # ---- window 2: attention B ----
            for _ in attn_stream(1):
                pass
            atsb_ctx.close()
            atps_ctx.close()
            qkv_ctx.close()

            # ---- window 3: MLP both slots, proj-A overlapped ----
            gp = stack.enter_context(tc.tile_pool(name="gp", bufs=1, side="right"))
            gT = gp.tile([128, 32, 2 * CH], bf16)
            prsb = stack.enter_context(tc.tile_pool(name="prsb", bufs=1))
            mlps_ctx = ExitStack()
            mlps = mlps_ctx.enter_context(
                tc.tile_pool(name="mlps", bufs=1, space="PSUM"))
            prps_ctx = ExitStack()
            prps = prps_ctx.enter_context(
                tc.tile_pool(name="prps", bufs=1, space="PSUM"))
            mlB_ctx = ExitStack()
            mlsbB = mlB_ctx.enter_context(tc.tile_pool(name="mlsbB", bufs=1))

            interleave(
                chain(ap_ln2_stream(0, mlsbB), fc_stream(0, gT, mlsbB)),
                ap_ln2_stream(1, mlsbB),
                16 / 48.0)
            interleave(
                fc_stream(1, gT, mlsbB),
                proj_stream(0, gT, prps, prsb),
                24 / 32.0)
            mlB_ctx.close()

            # ---- window 4: proj-B ----
            for _ in proj_stream(1, gT, prps, prsb):
                pass
            prps_ctx.close()
            mlps_ctx.close()

    nc.compile()
    return nc


def _host_prep(inputs):
    """Build the 8 per-core input maps."""
    x = np.asarray(inputs["x"], np.float32)
    ln1_g = np.asarray(inputs["ln1_g"], np.float32)
    ln1_b = np.asarray(inputs["ln1_b"], np.float32)
    ln2_g = np.asarray(inputs["ln2_g"], np.float32)
    ln2_b = np.asarray(inputs["ln2_b"], np.float32)

    # Fold LN1 gamma/beta into the QKV GEMM, and 1/sqrt(head_dim) into Q.
    w_attn_raw = np.asarray(inputs["w_attn"], np.float32)
    w_attn = (w_attn_raw * ln1_g[:, None]).copy()
    b_attn = (np.asarray(inputs["b_attn"], np.float32)
              + ln1_b @ w_attn_raw).copy()
    w_attn[:, :E] *= 0.125
    b_attn[:E] *= 0.125
    w_attn_bf = np.ascontiguousarray(w_attn.astype(BF))
    b_qk = np.ascontiguousarray(b_attn[:2 * E].reshape(2 * E, 1))
    b_v = np.ascontiguousarray(b_attn[2 * E:].reshape(1, E))

    # Fold LN2 gamma/beta into the FC GEMM.
    w_fc_raw = np.asarray(inputs["w_fc"], np.float32)
    w_fc = w_fc_raw * ln2_g[:, None]
    b_fc = np.asarray(inputs["b_fc"], np.float32) + ln2_b @ w_fc_raw

    w_ap_bf = np.ascontiguousarray(np.asarray(inputs["w_attnproj"], np.float32).astype(BF))
    w_fc_bf = np.ascontiguousarray(w_fc.astype(BF))
    w_pr_bf = np.ascontiguousarray(np.asarray(inputs["w_proj"], np.float32).astype(BF))
    col = lambda v: np.ascontiguousarray(np.asarray(v, np.float32).reshape(-1, 1))
    b_ap = col(inputs["b_attnproj"])
    b_fc = col(b_fc)
    b_pr = col(inputs["b_proj"])

    # static diagonal pair masks (bf16 0/1, applied post-exp):
    # within a 512-chunk, kv tile t allows query col j iff j >= t*128 + p.
    j = np.arange(CH)[None, :]
    p = np.arange(128)[:, None]
    m01 = [np.where(j >= t * 128 + p, 1.0, 0.0).astype(np.float32)
           for t in range(4)]
    pm = np.stack([np.concatenate([m01[0], m01[1]], axis=1),
                   np.concatenate([m01[2], m01[3]], axis=1)])
    pm_bf = np.ascontiguousarray(pm.astype(BF))

    ON = (1.0, 0.0)
    OFF = (0.0, NEG)
    in_maps = []
    perms = []
    for core in range(8):
        b = core // 2
        z = core % 2
        blocks = [0, 3, 1, 2] if z == 0 else [1, 2, 0, 3]
        perms.append(blocks)
        cols = np.concatenate([np.arange(c * CH, (c + 1) * CH) for c in blocks])
        xT = np.ascontiguousarray(x[b].T[:, cols])
        # slot A: driven block = O1 (perm pos 2); allowed iff block(O1) < block(A)
        sa = ON if blocks[2] < blocks[0] else OFF
        # slot B: driven = A, O1, O2 (perm pos 0, 2, 3) vs chunk B
        sbs = [ON if blocks[i] < blocks[1] else OFF for i in (0, 2, 3)]
        f = np.float32
        in_maps.append({
            "xT": xT,
            "w_attn": w_attn_bf, "b_qk": b_qk, "b_v": b_v,
            "w_ap": w_ap_bf, "b_ap": b_ap,
            "w_fc": w_fc_bf, "b_fc": b_fc, "w_proj": w_pr_bf, "b_proj": b_pr,
            "pmask": pm_bf,
            "sA_scale": np.full((128, 1), sa[0], f),
            "sA_bias": np.full((128, 1), sa[1], f),
            "sB_scale": np.ascontiguousarray(
                np.tile(np.array([[s for s, _ in sbs]], f), (128, 1))),
            "sB_bias": np.ascontiguousarray(
                np.tile(np.array([[bb for _, bb in sbs]], f), (128, 1))),
        })
    return in_maps, perms


def _run(inputs, trace=False):
    from concourse.bass_utils import run_bass_kernel_spmd

    if "nc" not in _CACHE:
        _CACHE["nc"] = _build_program()
    nc = _CACHE["nc"]
    in_maps, perms = _host_prep(inputs)
    res = run_bass_kernel_spmd(nc, in_maps, list(range(8)), trace=trace)
    x = np.asarray(inputs["x"], np.float32)
    out = np.empty_like(x)
    for core in range(8):
        b = core // 2
        blocks = perms[core]
        oT = res.results[core]["outT"]
        cA, cB = blocks[0], blocks[1]
        out[b, cA * CH:(cA + 1) * CH, :] = oT[:, 0:CH].T
        out[b, cB * CH:(cB + 1) * CH, :] = oT[:, CH:2 * CH].T
    return out, res


def kernel(**inputs) -> np.ndarray:
    out, _ = _run(inputs, trace=False)
    return out


# revision 36
# speedup vs baseline: 1.0084x; 1.0084x over previous
"""GPT block (LN -> causal MHA -> LN -> MLP) on 8 TRN2 NeuronCores.

Sharding: each core owns one (batch, query-chunk-pair). B=4 batches x 2
chunk-pairs = 8 cores. Chunk pairs are zig-zag ({0,3} / {1,2}) over four
512-row chunks of T=2048 so attention work balances. Each core recomputes
K/V for the full sequence locally (no collectives), runs flash-style
attention for its 1024 query rows, then the MLP for the same rows.

All activations live feature-on-partition ("transposed"); the host
pre-transposes x and assembles the output. Per-core causality is handled
with a block permutation of the sequence; full-block allow/deny is driven
by per-core exp scale/bias inputs (exp(0*s - 1e9) = 0 kills forbidden
blocks) and the partial diagonal blocks by static 0/1 bf16 masks applied
post-exp.

Schedule (v3): one software-pipelined stream so the PE never idles on the
Act engine's exp:
  LN1 (chunk-pipelined) -> QKV (minus K/V tiles 12-15) ->
  [attention slot A || deferred K/V GEMMs] ->
  [attention slot B || attnproj-A + LN2-A] ->
  [FC-A, proj-A || attnproj-B, LN2-B, FC-B] -> proj-B.
Exp is batched over two 512-col score tiles in adjacent PSUM banks; LN
gamma/beta ride the next GEMM's weights (host-folded); LN1 stats matmuls
run in float32r (no bf16 cast); softmax denominators use
reciprocal_approx_fast + Pool-engine partition broadcast.
"""

import numpy as np
import ml_dtypes

BF = ml_dtypes.bfloat16

E = 1024          # embedding
T = 2048          # sequence
B = 4             # batch
NH = 16           # heads
D = 64            # head dim
HID = 4096        # mlp hidden
KT = E // 128     # k-tiles over embedding (8)
CH = 512          # chunk rows
NEG = -1.0e9
EPS = 1e-5

_CACHE = {}


def _build_program():
    import concourse.bass as bass
    import concourse.tile as tile
    from concourse import bacc, mybir

    f32 = mybir.dt.float32
    f32r = mybir.dt.float32r
    bf16 = mybir.dt.bfloat16
    AF = mybir.ActivationFunctionType
    ALU = mybir.AluOpType

    nc = bacc.Bacc()

    xT_d = nc.declare_dram_parameter("xT", [E, T], f32, isOutput=False)
    w_attn_d = nc.declare_dram_parameter("w_attn", [E, 3 * E], bf16, isOutput=False)
    b_qk_d = nc.declare_dram_parameter("b_qk", [2 * E, 1], f32, isOutput=False)
    b_v_d = nc.declare_dram_parameter("b_v", [1, E], f32, isOutput=False)
    w_ap_d = nc.declare_dram_parameter("w_ap", [E, E], bf16, isOutput=False)
    b_ap_d = nc.declare_dram_parameter("b_ap", [E, 1], f32, isOutput=False)
    w_fc_d = nc.declare_dram_parameter("w_fc", [E, HID], bf16, isOutput=False)
    b_fc_d = nc.declare_dram_parameter("b_fc", [HID, 1], f32, isOutput=False)
    w_pr_d = nc.declare_dram_parameter("w_proj", [HID, E], bf16, isOutput=False)
    b_pr_d = nc.declare_dram_parameter("b_proj", [E, 1], f32, isOutput=False)
    pm_d = nc.declare_dram_parameter("pmask", [2, 128, 2 * CH], bf16, isOutput=False)
    sA_s_d = nc.declare_dram_parameter("sA_scale", [128, 1], f32, isOutput=False)
    sA_b_d = nc.declare_dram_parameter("sA_bias", [128, 1], f32, isOutput=False)
    sB_s_d = nc.declare_dram_parameter("sB_scale", [128, 3], f32, isOutput=False)
    sB_b_d = nc.declare_dram_parameter("sB_bias", [128, 3], f32, isOutput=False)
    out_d = nc.declare_dram_parameter("outT", [E, 2 * CH], f32, isOutput=True)

    wT = w_attn_d.rearrange("(k p) n -> p k n", p=128)
    wfcT = w_fc_d.rearrange("(k p) n -> p k n", p=128)
    wapT = w_ap_d.rearrange("(k p) n -> p k n", p=128)
    wprT = w_pr_d.rearrange("(k p) n -> p k n", p=128)
    xTr = xT_d.rearrange("(k p) n -> p k n", p=128)

    with tile.TileContext(nc) as tc:
        from contextlib import ExitStack

        stack = ExitStack()
        with stack:
            # ---- long-lived left-side pools (LIFO nesting) ----
            const = stack.enter_context(tc.tile_pool(name="const", bufs=1))
            aBp = stack.enter_context(tc.tile_pool(name="aBp", bufs=1))
            h2p = stack.enter_context(tc.tile_pool(name="h2p", bufs=1))
            # ---- right-side ----
            x2p = stack.enter_context(tc.tile_pool(name="x2p", bufs=1, side="right"))

            ones_col_bf = const.tile([128, 1], bf16)
            nc.vector.memset(ones_col_bf[:], 1.0)
            ones_col_f = const.tile([128, 1], f32)
            nc.vector.memset(ones_col_f[:], 1.0)
            eps_t = const.tile([1, 1], f32)
            nc.vector.memset(eps_t[:], EPS)

            pmask = const.tile([128, 2, 2 * CH], bf16)
            nc.sync.dma_start(pmask[:], pm_d.rearrange("v p n -> p v n"))
            sA_s = const.tile([128, 1], f32)
            nc.sync.dma_start(sA_s[:], sA_s_d[:])
            sA_b = const.tile([128, 1], f32)
            nc.sync.dma_start(sA_b[:], sA_b_d[:])
            sB_s = const.tile([128, 3], f32)
            nc.sync.dma_start(sB_s[:], sB_s_d[:])
            sB_b = const.tile([128, 3], f32)
            nc.sync.dma_start(sB_b[:], sB_b_d[:])

            b_qk = const.tile([128, 16, 1], f32)
            nc.sync.dma_start(b_qk[:], b_qk_d.rearrange("(k p) o -> p k o", p=128))
            b_v = const.tile([1, E], f32)
            nc.sync.dma_start(b_v[:], b_v_d[:])
            b_ap = const.tile([128, KT, 1], f32)
            nc.sync.dma_start(b_ap[:], b_ap_d.rearrange("(k p) o -> p k o", p=128))
            b_fc = const.tile([128, 32, 1], f32)
            nc.sync.dma_start(b_fc[:], b_fc_d.rearrange("(k p) o -> p k o", p=128))
            b_pr = const.tile([128, KT, 1], f32)
            nc.sync.dma_start(b_pr[:], b_pr_d.rearrange("(k p) o -> p k o", p=128))

            attnTB = aBp.tile([128, KT, CH], bf16)
            h2T = h2p.tile([128, KT, 2 * CH], bf16)
            x2T = x2p.tile([128, KT, 2 * CH], bf16)

            # window-scoped pools (opened/closed in LIFO order)
            aAp = stack.enter_context(tc.tile_pool(name="aAp", bufs=1))
            attnTA = aAp.tile([128, KT, CH], bf16)
            attnTs = [attnTA, attnTB]

            atsb_ctx = ExitStack()
            atsb = atsb_ctx.enter_context(tc.tile_pool(name="atsb", bufs=4))

            # ---------------- Phase 1: LN1 (chunk-pipelined) ----------------
            ln1_ctx = ExitStack()
            ln1 = ln1_ctx.enter_context(tc.tile_pool(name="ln1", bufs=1))
            ln1_t = ln1.tile([128, KT, T], bf16)

            with tc.tile_pool(name="ln1ps", bufs=1, space="PSUM") as lnps, \
                 tc.tile_pool(name="ln1sb", bufs=1) as lnsb:
                state = {}

                def ln1_stats(ch):
                    xt = lnsb.tile([128, KT, CH], f32, tag="xin", bufs=2,
                                   name=f"l1x{ch}")
                    nc.sync.dma_start(xt[:], xTr[:, :, ch * CH:(ch + 1) * CH])
                    mu_ps = lnps.tile([1, CH], f32, tag="stat", bufs=4,
                                      name=f"l1mu{ch}")
                    ss_ps = lnps.tile([1, CH], f32, tag="stat", bufs=4,
                                      name=f"l1ss{ch}")
                    for kt in range(KT):
                        xbf = lnsb.tile([128, CH], bf16, tag="xbf", bufs=3,
                                        name=f"l1xb{ch}_{kt}")
                        if kt % 2 == 0:
                            nc.scalar.activation(xbf[:], xt[:, kt, :], AF.Copy)
                        else:
                            nc.vector.tensor_copy(xbf[:], xt[:, kt, :])
                        sq = lnsb.tile([128, CH], bf16, tag="sq", bufs=3,
                                       name=f"l1sq{ch}_{kt}")
                        nc.scalar.square(sq[:], xbf[:])
                        nc.tensor.matmul(mu_ps[:], ones_col_bf[:], xbf[:],
                                         start=(kt == 0), stop=(kt == KT - 1))
                        nc.tensor.matmul(ss_ps[:], ones_col_bf[:], sq[:],
                                         start=(kt == 0), stop=(kt == KT - 1))
                    state[ch] = (xt, mu_ps, ss_ps)

                def ln1_norm(ch):
                    xt, mu_ps, ss_ps = state.pop(ch)
                    mu = lnsb.tile([1, CH], f32, tag="row", bufs=8)
                    nc.scalar.activation(mu[:], mu_ps[:], AF.Copy,
                                         scale=1.0 / E)
                    ms = lnsb.tile([1, CH], f32, tag="row", bufs=8)
                    nc.scalar.activation(ms[:], ss_ps[:], AF.Copy,
                                         scale=1.0 / E)
                    msq = lnsb.tile([1, CH], f32, tag="row", bufs=8)
                    nc.scalar.square(msq[:], mu[:])
                    var = lnsb.tile([1, CH], f32, tag="row", bufs=8)
                    nc.vector.tensor_sub(var[:], ms[:], msq[:])
                    sd = lnsb.tile([1, CH], f32, tag="row", bufs=8)
                    nc.scalar.activation(sd[:], var[:], AF.Sqrt,
                                         bias=eps_t[:])
                    rstd = lnsb.tile([1, CH], f32, tag="row", bufs=8)
                    nc.vector.reciprocal_approx_fast(rstd[:], sd[:])
                    nmr = lnsb.tile([1, CH], f32, tag="row", bufs=8)
                    nc.vector.scalar_tensor_tensor(
                        nmr[:], mu[:], -1.0, rstd[:], ALU.mult, ALU.mult)
                    a_bc = lnsb.tile([128, CH], f32, tag="bc", bufs=4)
                    nc.gpsimd.partition_broadcast(a_bc[:], rstd[:])
                    c_bc = lnsb.tile([128, CH], f32, tag="bc", bufs=4)
                    nc.gpsimd.partition_broadcast(c_bc[:], nmr[:])
                    for kt in range(KT):
                        t1 = lnsb.tile([128, CH], f32, tag="t1", bufs=3)
                        nc.vector.tensor_mul(t1[:], xt[:, kt, :], a_bc[:])
                        nc.vector.tensor_add(
                            ln1_t[:, kt, ch * CH:(ch + 1) * CH], t1[:], c_bc[:])

                for ch in range(5):
                    if ch < 4:
                        ln1_stats(ch)
                    if ch >= 1:
                        ln1_norm(ch - 1)

            # ---------------- Phase 2: QKV (minus deferred) ----------------
            qA_ctx = ExitStack()
            qAp = qA_ctx.enter_context(tc.tile_pool(name="qAp", bufs=1))
            qTA = qAp.tile([128, KT, CH], bf16)
            qkv_ctx = ExitStack()
            qkvp = qkv_ctx.enter_context(
                tc.tile_pool(name="qkvp", bufs=1, side="right"))
            qTB = qkvp.tile([128, KT, CH], bf16)
            qTs = [qTA, qTB]
            kT = qkvp.tile([128, KT, T], bf16)
            v_aug = qkvp.tile([128, 16, NH * 65], bf16)
            v4 = v_aug.rearrange("p m (h w) -> p m h w", h=NH)

            with tc.tile_pool(name="wqk", bufs=2) as wqk_pool, \
                 tc.tile_pool(name="qkps", bufs=4, space="PSUM") as qkps:
                for g in range(8):
                    panel = wqk_pool.tile([128, KT, 256], bf16, tag="w")
                    nc.sync.dma_start(panel[:], wT[:, :, g * 256:(g + 1) * 256])
                    for mm in range(2):
                        mt = 2 * g + mm
                        is_q = mt < 8
                        n_chunks = 2 if is_q else 4
                        dt_idx = mt if is_q else mt - 8
                        for nq in range(n_chunks):
                            ps = qkps.tile([128, CH], f32, tag="ps")
                            for kt in range(KT):
                                nc.tensor.matmul(
                                    ps[:], panel[:, kt, mm * 128:(mm + 1) * 128],
                                    ln1_t[:, kt, nq * CH:(nq + 1) * CH],
                                    start=(kt == 0), stop=(kt == KT - 1))
                            dst_ap = (qTs[nq][:, dt_idx, :] if is_q else
                                      kT[:, dt_idx, nq * CH:(nq + 1) * CH])
                            nc.scalar.activation(
                                dst_ap, ps[:],
                                AF.Identity, bias=b_qk[:, mt, 0:1])
                for g in range(2):
                    panel = wqk_pool.tile([128, KT, CH], bf16, tag="wv",
                                          bufs=1)
                    nc.sync.dma_start(
                        panel[:], wT[:, :, 2 * E + g * CH:2 * E + (g + 1) * CH])
                    bv_sb = wqk_pool.tile([128, CH], f32, tag="bvs", bufs=1)
                    nc.gpsimd.partition_broadcast(
                        bv_sb[:], b_v[:, g * CH:(g + 1) * CH])
                    bv_sb3 = bv_sb.rearrange("p (h w) -> p h w", h=8)
                    for mv in range(16):
                        ps = qkps.tile([128, CH], f32, tag="ps")
                        for kt in range(KT):
                            nc.tensor.matmul(
                                ps[:], ln1_t[:, kt, mv * 128:(mv + 1) * 128],
                                panel[:, kt, :],
                                start=(kt == 0), stop=(kt == KT - 1))
                        ps3 = ps.rearrange("p (h w) -> p h w", h=8)
                        nc.vector.tensor_add(
                            v4[:, mv, g * 8:(g + 1) * 8, 0:64], ps3[:], bv_sb3[:])
                for mv in range(16):
                    nc.vector.memset(v4[:, mv, :, 64:65], 1.0)

            # attention PSUM pool: s 3x2 banks + av 2 = 8 banks
            atps_ctx = ExitStack()
            atps = atps_ctx.enter_context(
                tc.tile_pool(name="atps", bufs=1, space="PSUM"))

            # ============ attention machinery ============
            pairs_a = [(0, 1, ("diag", 0)), (2, 3, ("diag", 1)),
                       (8, 9, ("drv", "A", 0)), (10, 11, ("drv", "A", 0))]
            pairs_b = [(4, 5, ("diag", 0)), (6, 7, ("diag", 1)),
                       (0, 1, ("drv", "B", 0)), (2, 3, ("drv", "B", 0)),
                       (8, 9, ("drv", "B", 1)), (10, 11, ("drv", "B", 1)),
                       (12, 13, ("drv", "B", 2)), (14, 15, ("drv", "B", 2))]

            avs = {}
            ess = {}

            def emit_s_exp(work, i):
                h, slot, j, npairs, (t0, t1, mk) = work[i]
                ktf = h // 2
                ro = (h % 2) * 64
                s_ps = atps.tile([128, 2 * CH], f32, tag="s", bufs=3)
                for k, t in enumerate((t0, t1)):
                    nc.tensor.matmul(
                        s_ps[:, k * CH:(k + 1) * CH],
                        kT[ro:ro + 64, ktf, t * 128:(t + 1) * 128],
                        qTs[slot][ro:ro + 64, ktf, :],
                        start=True, stop=True)
                es = atsb.tile([128, 2 * CH], bf16, tag="es", bufs=4)
                if mk[0] == "diag":
                    nc.scalar.activation(es[:], s_ps[:], AF.Exp)
                    nc.vector.tensor_mul(es[:], es[:], pmask[:, mk[1], :])
                else:
                    sc = sA_s if mk[1] == "A" else sB_s
                    bi = sA_b if mk[1] == "A" else sB_b
                    idx = mk[2]
                    nc.scalar.activation(
                        es[:], s_ps[:], AF.Exp,
                        bias=bi[:, idx:idx + 1],
                        scale=sc[:, idx:idx + 1])
                ess[(h, slot, j)] = es

            def emit_pv(work, i):
                h, slot, j, npairs, (t0, t1, mk) = work[i]
                if j == 0:
                    avs[(h, slot)] = atps.tile([65, CH], f32, tag="av",
                                               bufs=2, name=f"av{h}_{slot}")
                out_ps = avs[(h, slot)]
                es = ess.pop((h, slot, j))
                for k, t in enumerate((t0, t1)):
                    nc.tensor.matmul(
                        out_ps[:], v_aug[:, t, h * 65:(h + 1) * 65],
                        es[:, k * CH:(k + 1) * CH],
                        start=(j == 0 and k == 0),
                        stop=(j == npairs - 1 and k == 1))
                if j == npairs - 1:
                    ktf = h // 2
                    ro = (h % 2) * 64
                    den = atsb.tile([1, CH], f32, tag="dn", bufs=2)
                    nc.vector.tensor_copy(den[:], out_ps[64:65, :])
                    rec = atsb.tile([1, CH], f32, tag="rc", bufs=2)
                    nc.vector.reciprocal_approx_fast(rec[:], den[:])
                    bc64 = atsb.tile([64, CH], f32, tag="bcr", bufs=2)
                    nc.gpsimd.partition_broadcast(bc64[:], rec[:])
                    nc.vector.tensor_mul(
                        attnTs[slot][ro:ro + 64, ktf, :], out_ps[0:64, :],
                        bc64[:])

            def attn_stream(slot):
                seq = pairs_a if slot == 0 else pairs_b
                work = []
                for h in range(NH):
                    for j, p in enumerate(seq):
                        work.append((h, slot, j, len(seq), p))
                LEAD = 2
                for i in range(len(work) + LEAD):
                    if i < len(work):
                        emit_s_exp(work, i)
                    if i >= LEAD:
                        emit_pv(work, i - LEAD)
                    yield

            def ap_ln2_stream(slot, mlsb):
                """attnproj + residual -> LN2 for one slot."""
                nq = slot
                for m in range(KT):
                    wpan = mlsb.tile([128, KT, 128], bf16, tag="wap", bufs=2,
                                     name=f"wap{slot}_{m}")
                    nc.sync.dma_start(
                        wpan[:], wapT[:, :, m * 128:(m + 1) * 128])
                    ps = mlps.tile([128, CH], f32, tag="ps", bufs=2,
                                   name=f"ap{slot}_{m}")
                    for kt in range(KT):
                        nc.tensor.matmul(
                            ps[:], wpan[:, kt, :],
                            attnTs[slot][:, kt, :],
                            start=(kt == 0), stop=(kt == KT - 1))
                    xq = mlsb.tile([128, CH], f32, tag="xq", bufs=2,
                                   name=f"xq{slot}_{m}")
                    nc.sync.dma_start(
                        xq[:], xT_d[m * 128:(m + 1) * 128,
                                    nq * CH:(nq + 1) * CH])
                    nc.vector.scalar_tensor_tensor(
                        x2T[:, m, nq * CH:(nq + 1) * CH], ps[:],
                        b_ap[:, m, 0:1], xq[:], ALU.add, ALU.add)
                    yield
                # LN2 (stats borrow partitions 0/32 of a [128, CH] psum tile;
                # x2 is bf16 so the stats matmuls run in bf16 directly)
                stat_t = mlps.tile([128, CH], f32, tag="ps", bufs=2,
                                   name=f"l2s{slot}")
                mu_ps = stat_t[0:1, :]
                ss_ps = stat_t[32:33, :]
                src = x2T[:, :, nq * CH:(nq + 1) * CH]
                for kt in range(KT):
                    sq = mlsb.tile([128, CH], bf16, tag="sq2", bufs=2,
                                   name=f"l2sq{slot}_{kt}")
                    nc.scalar.square(sq[:], src[:, kt, :])
                    nc.tensor.matmul(mu_ps, ones_col_bf[:], src[:, kt, :],
                                     start=(kt == 0), stop=(kt == KT - 1))
                    nc.tensor.matmul(ss_ps, ones_col_bf[:], sq[:],
                                     start=(kt == 0), stop=(kt == KT - 1))
                    if kt % 2 == 1:
                        yield
                mu = mlsb.tile([1, CH], f32, tag="row2", bufs=7)
                nc.scalar.activation(mu[:], mu_ps, AF.Copy, scale=1.0 / E)
                ms = mlsb.tile([1, CH], f32, tag="row2", bufs=7)
                nc.scalar.activation(ms[:], ss_ps, AF.Copy, scale=1.0 / E)
                msq = mlsb.tile([1, CH], f32, tag="row2", bufs=7)
                nc.scalar.square(msq[:], mu[:])
                var = mlsb.tile([1, CH], f32, tag="row2", bufs=7)
                nc.vector.tensor_sub(var[:], ms[:], msq[:])
                sd = mlsb.tile([1, CH], f32, tag="row2", bufs=7)
                nc.scalar.activation(sd[:], var[:], AF.Sqrt, bias=eps_t[:])
                rstd = mlsb.tile([1, CH], f32, tag="row2", bufs=7)
                nc.vector.reciprocal_approx_fast(rstd[:], sd[:])
                nmr = mlsb.tile([1, CH], f32, tag="row2", bufs=7)
                nc.vector.scalar_tensor_tensor(
                    nmr[:], mu[:], -1.0, rstd[:], ALU.mult, ALU.mult)
                a_bc = mlsb.tile([128, CH], f32, tag="bc2", bufs=2)
                nc.gpsimd.partition_broadcast(a_bc[:], rstd[:])
                c_bc = mlsb.tile([128, CH], f32, tag="bc2", bufs=2)
                nc.gpsimd.partition_broadcast(c_bc[:], nmr[:])
                for kt in range(KT):
                    t1 = mlsb.tile([128, CH], f32, tag="t12", bufs=2)
                    nc.vector.tensor_mul(t1[:], src[:, kt, :], a_bc[:])
                    nc.vector.tensor_add(
                        h2T[:, kt, nq * CH:(nq + 1) * CH], t1[:], c_bc[:])
                    if kt % 2 == 1:
                        yield

            def fc_stream(slot, g_t, mlsb):
                nq = slot
                for mg in range(8):
                    panel = mlsb.tile([128, KT, CH], bf16, tag="wfc", bufs=1,
                                      name=f"fc{slot}_{mg}")
                    nc.sync.dma_start(
                        panel[:], wfcT[:, :, mg * CH:(mg + 1) * CH])
                    for mm in range(4):
                        ps = mlps.tile([128, CH], f32, tag="ps", bufs=2,
                                       name=f"fc{slot}_{mg}_{mm}")
                        for kt in range(KT):
                            nc.tensor.matmul(
                                ps[:], panel[:, kt, mm * 128:(mm + 1) * 128],
                                h2T[:, kt, nq * CH:(nq + 1) * CH],
                                start=(kt == 0), stop=(kt == KT - 1))
                        mt = mg * 4 + mm
                        nc.scalar.activation(
                            g_t[:, mt, nq * CH:(nq + 1) * CH], ps[:],
                            AF.Gelu, bias=b_fc[:, mt, 0:1])
                        yield

            def proj_stream(slot, g_t, prps, prsb):
                nq = slot
                for half in range(2):
                    ms = range(half * 4, (half + 1) * 4)
                    pss = [prps.tile([128, CH], f32, tag="ps", bufs=4,
                                     name=f"pr{slot}_{half}_{m}")
                           for m in ms]
                    for kg in range(8):
                        panel = prsb.tile([128, 4, E], bf16, tag="w", bufs=2,
                                          name=f"prw{slot}_{half}_{kg}")
                        nc.sync.dma_start(
                            panel[:], wprT[:, 4 * kg:4 * kg + 4, :])
                        for kk in range(4):
                            kt = kg * 4 + kk
                            for mi, m in enumerate(ms):
                                nc.tensor.matmul(
                                    pss[mi][:],
                                    panel[:, kk, m * 128:(m + 1) * 128],
                                    g_t[:, kt, nq * CH:(nq + 1) * CH],
                                    start=(kt == 0), stop=(kt == 31),
                                    skip_group_check=True)
                        yield
                    for mi, m in enumerate(ms):
                        ot = prsb.tile([128, CH], f32, tag="ot", bufs=3,
                                       name=f"ot{slot}_{half}_{m}")
                        nc.vector.scalar_tensor_tensor(
                            ot[:], pss[mi][:], b_pr[:, m, 0:1],
                            x2T[:, m, nq * CH:(nq + 1) * CH],
                            ALU.add, ALU.add)
                        nc.sync.dma_start(
                            out_d[m * 128:(m + 1) * 128,
                                  nq * CH:(nq + 1) * CH],
                            ot[:])
                        yield

            def chain(*gens):
                for g in gens:
                    for _ in g:
                        yield

            def interleave(main, filler, ratio):
                budget = 0.0
                for _ in main:
                    budget += ratio
                    while budget >= 1.0 and filler is not None:
                        budget -= 1.0
                        try:
                            next(filler)
                        except StopIteration:
                            filler = None
                if filler is not None:
                    for _ in filler:
                        pass

            # ---- window 1: attention A ----
            for _ in attn_stream(0):
                pass
            qA_ctx.close()
            ln1_ctx.close()

            # ---- window 2: attention B ----
            for _ in attn_stream(1):
                pass
            atsb_ctx.close()
            atps_ctx.close()
            qkv_ctx.close()

            # ---- window 3: MLP both slots, proj-A overlapped ----
            gp = stack.enter_context(tc.tile_pool(name="gp", bufs=1, side="right"))
            gT = gp.tile([128, 32, 2 * CH], bf16)
            prsb = stack.enter_context(tc.tile_pool(name="prsb", bufs=1))
            mlps_ctx = ExitStack()
            mlps = mlps_ctx.enter_context(
                tc.tile_pool(name="mlps", bufs=1, space="PSUM"))
            prps_ctx = ExitStack()
            prps = prps_ctx.enter_context(
                tc.tile_pool(name="prps", bufs=1, space="PSUM"))
            mlB_ctx = ExitStack()
            mlsbB = mlB_ctx.enter_context(tc.tile_pool(name="mlsbB", bufs=1))

            interleave(
                chain(ap_ln2_stream(0, mlsbB), fc_stream(0, gT, mlsbB)),
                ap_ln2_stream(1, mlsbB),
                16 / 48.0)
            interleave(
                fc_stream(1, gT, mlsbB),
                proj_stream(0, gT, prps, prsb),
                24 / 32.0)
            mlB_ctx.close()

            # ---- window 4: proj-B ----
            for _ in proj_stream(1, gT, prps, prsb):
                pass
            prps_ctx.close()
            mlps_ctx.close()

    nc.compile()
    return nc


def _host_prep(inputs):
    """Build the 8 per-core input maps."""
    x = np.asarray(inputs["x"], np.float32)
    ln1_g = np.asarray(inputs["ln1_g"], np.float32)
    ln1_b = np.asarray(inputs["ln1_b"], np.float32)
    ln2_g = np.asarray(inputs["ln2_g"], np.float32)
    ln2_b = np.asarray(inputs["ln2_b"], np.float32)

    # Fold LN1 gamma/beta into the QKV GEMM, and 1/sqrt(head_dim) into Q.
    w_attn_raw = np.asarray(inputs["w_attn"], np.float32)
    w_attn = (w_attn_raw * ln1_g[:, None]).copy()
    b_attn = (np.asarray(inputs["b_attn"], np.float32)
              + ln1_b @ w_attn_raw).copy()
    w_attn[:, :E] *= 0.125
    b_attn[:E] *= 0.125
    w_attn_bf = np.ascontiguousarray(w_attn.astype(BF))
    b_qk = np.ascontiguousarray(b_attn[:2 * E].reshape(2 * E, 1))
    b_v = np.ascontiguousarray(b_attn[2 * E:].reshape(1, E))

    # Fold LN2 gamma/beta into the FC GEMM.
    w_fc_raw = np.asarray(inputs["w_fc"], np.float32)
    w_fc = w_fc_raw * ln2_g[:, None]
    b_fc = np.asarray(inputs["b_fc"], np.float32) + ln2_b @ w_fc_raw

    w_ap_bf = np.ascontiguousarray(np.asarray(inputs["w_attnproj"], np.float32).astype(BF))
    w_fc_bf = np.ascontiguousarray(w_fc.astype(BF))
    w_pr_bf = np.ascontiguousarray(np.asarray(inputs["w_proj"], np.float32).astype(BF))
    col = lambda v: np.ascontiguousarray(np.asarray(v, np.float32).reshape(-1, 1))
    b_ap = col(inputs["b_attnproj"])
    b_fc = col(b_fc)
    b_pr = col(inputs["b_proj"])

    # static diagonal pair masks (bf16 0/1, applied post-exp):
    # within a 512-chunk, kv tile t allows query col j iff j >= t*128 + p.
    j = np.arange(CH)[None, :]
    p = np.arange(128)[:, None]
    m01 = [np.where(j >= t * 128 + p, 1.0, 0.0).astype(np.float32)
           for t in range(4)]
    pm = np.stack([np.concatenate([m01[0], m01[1]], axis=1),
                   np.concatenate([m01[2], m01[3]], axis=1)])
    pm_bf = np.ascontiguousarray(pm.astype(BF))

    ON = (1.0, 0.0)
    OFF = (0.0, NEG)
    in_maps = []
    perms = []
    for core in range(8):
        b = core // 2
        z = core % 2
        blocks = [0, 3, 1, 2] if z == 0 else [1, 2, 0, 3]
        perms.append(blocks)
        cols = np.concatenate([np.arange(c * CH, (c + 1) * CH) for c in blocks])
        xT = np.ascontiguousarray(x[b].T[:, cols])
        # slot A: driven block = O1 (perm pos 2); allowed iff block(O1) < block(A)
        sa = ON if blocks[2] < blocks[0] else OFF
        # slot B: driven = A, O1, O2 (perm pos 0, 2, 3) vs chunk B
        sbs = [ON if blocks[i] < blocks[1] else OFF for i in (0, 2, 3)]
        f = np.float32
        in_maps.append({
            "xT": xT,
            "w_attn": w_attn_bf, "b_qk": b_qk, "b_v": b_v,
            "w_ap": w_ap_bf, "b_ap": b_ap,
            "w_fc": w_fc_bf, "b_fc": b_fc, "w_proj": w_pr_bf, "b_proj": b_pr,
            "pmask": pm_bf,
            "sA_scale": np.full((128, 1), sa[0], f),
            "sA_bias": np.full((128, 1), sa[1], f),
            "sB_scale": np.ascontiguousarray(
                np.tile(np.array([[s for s, _ in sbs]], f), (128, 1))),
            "sB_bias": np.ascontiguousarray(
                np.tile(np.array([[bb for _, bb in sbs]], f), (128, 1))),
        })
    return in_maps, perms


def _run(inputs, trace=False):
    from concourse.bass_utils import run_bass_kernel_spmd

    if "nc" not in _CACHE:
        _CACHE["nc"] = _build_program()
    nc = _CACHE["nc"]
    in_maps, perms = _host_prep(inputs)
    res = run_bass_kernel_spmd(nc, in_maps, list(range(8)), trace=trace)
    x = np.asarray(inputs["x"], np.float32)
    out = np.empty_like(x)
    for core in range(8):
        b = core // 2
        blocks = perms[core]
        oT = res.results[core]["outT"]
        cA, cB = blocks[0], blocks[1]
        out[b, cA * CH:(cA + 1) * CH, :] = oT[:, 0:CH].T
        out[b, cB * CH:(cB + 1) * CH, :] = oT[:, CH:2 * CH].T
    return out, res


def kernel(**inputs) -> np.ndarray:
    out, _ = _run(inputs, trace=False)
    return out


# revision 39
# speedup vs baseline: 1.0311x; 1.0226x over previous
"""GPT block (LN -> causal MHA -> LN -> MLP) on 8 TRN2 NeuronCores.

Sharding: each core owns one (batch, query-chunk-pair). B=4 batches x 2
chunk-pairs = 8 cores. Chunk pairs are zig-zag ({0,3} / {1,2}) over four
512-row chunks of T=2048 so attention work balances. Each core recomputes
K/V for the full sequence locally (no collectives), runs flash-style
attention for its 1024 query rows, then the MLP for the same rows.

All activations live feature-on-partition ("transposed"); the host
pre-transposes x and assembles the output. Per-core causality is handled
with a block permutation of the sequence; full-block allow/deny is driven
by per-core exp scale/bias inputs (exp(0*s - 1e9) = 0 kills forbidden
blocks) and the partial diagonal blocks by static 0/1 bf16 masks applied
post-exp.

Schedule (v3): one software-pipelined stream so the PE never idles on the
Act engine's exp:
  LN1 (chunk-pipelined) -> QKV (minus K/V tiles 12-15) ->
  [attention slot A || deferred K/V GEMMs] ->
  [attention slot B || attnproj-A + LN2-A] ->
  [FC-A, proj-A || attnproj-B, LN2-B, FC-B] -> proj-B.
Exp is batched over two 512-col score tiles in adjacent PSUM banks; LN
gamma/beta ride the next GEMM's weights (host-folded); LN1 stats matmuls
run in float32r (no bf16 cast); softmax denominators use
reciprocal_approx_fast + Pool-engine partition broadcast.
"""

import numpy as np
import ml_dtypes

BF = ml_dtypes.bfloat16

E = 1024          # embedding
T = 2048          # sequence
B = 4             # batch
NH = 16           # heads
D = 64            # head dim
HID = 4096        # mlp hidden
KT = E // 128     # k-tiles over embedding (8)
CH = 512          # chunk rows
NEG = -1.0e9
EPS = 1e-5

_CACHE = {}


def _build_program():
    import concourse.bass as bass
    import concourse.tile as tile
    from concourse import bacc, mybir

    f32 = mybir.dt.float32
    f32r = mybir.dt.float32r
    bf16 = mybir.dt.bfloat16
    AF = mybir.ActivationFunctionType
    ALU = mybir.AluOpType

    nc = bacc.Bacc()

    xT_d = nc.declare_dram_parameter("xT", [E, T], f32, isOutput=False)
    w_attn_d = nc.declare_dram_parameter("w_attn", [E, 3 * E], bf16, isOutput=False)
    b_qk_d = nc.declare_dram_parameter("b_qk", [2 * E, 1], f32, isOutput=False)
    b_v_d = nc.declare_dram_parameter("b_v", [1, E], f32, isOutput=False)
    w_ap_d = nc.declare_dram_parameter("w_ap", [E, E], bf16, isOutput=False)
    b_ap_d = nc.declare_dram_parameter("b_ap", [E, 1], f32, isOutput=False)
    w_fc_d = nc.declare_dram_parameter("w_fc", [E, HID], bf16, isOutput=False)
    b_fc_d = nc.declare_dram_parameter("b_fc", [HID, 1], f32, isOutput=False)
    w_pr_d = nc.declare_dram_parameter("w_proj", [HID, E], bf16, isOutput=False)
    b_pr_d = nc.declare_dram_parameter("b_proj", [E, 1], f32, isOutput=False)
    pm_d = nc.declare_dram_parameter("pmask", [2, 128, 2 * CH], bf16, isOutput=False)
    sA_s_d = nc.declare_dram_parameter("sA_scale", [128, 1], f32, isOutput=False)
    sA_b_d = nc.declare_dram_parameter("sA_bias", [128, 1], f32, isOutput=False)
    sB_s_d = nc.declare_dram_parameter("sB_scale", [128, 3], f32, isOutput=False)
    sB_b_d = nc.declare_dram_parameter("sB_bias", [128, 3], f32, isOutput=False)
    out_d = nc.declare_dram_parameter("outT", [E, 2 * CH], f32, isOutput=True)

    wT = w_attn_d.rearrange("(k p) n -> p k n", p=128)
    wfcT = w_fc_d.rearrange("(k p) n -> p k n", p=128)
    wapT = w_ap_d.rearrange("(k p) n -> p k n", p=128)
    wprT = w_pr_d.rearrange("(k p) n -> p k n", p=128)
    xTr = xT_d.rearrange("(k p) n -> p k n", p=128)

    with tile.TileContext(nc) as tc:
        from contextlib import ExitStack

        stack = ExitStack()
        with stack:
            # ---- long-lived left-side pools (LIFO nesting) ----
            const = stack.enter_context(tc.tile_pool(name="const", bufs=1))
            aBp = stack.enter_context(tc.tile_pool(name="aBp", bufs=1))
            h2p = stack.enter_context(tc.tile_pool(name="h2p", bufs=1))
            # ---- right-side ----
            x2p = stack.enter_context(tc.tile_pool(name="x2p", bufs=1, side="right"))

            ones_col_bf = const.tile([128, 1], bf16)
            nc.vector.memset(ones_col_bf[:], 1.0)
            ones_col_f = const.tile([128, 1], f32)
            nc.vector.memset(ones_col_f[:], 1.0)
            eps_t = const.tile([1, 1], f32)
            nc.vector.memset(eps_t[:], EPS)

            pmask = const.tile([128, 2, 2 * CH], bf16)
            nc.sync.dma_start(pmask[:], pm_d.rearrange("v p n -> p v n"))
            sA_s = const.tile([128, 1], f32)
            nc.sync.dma_start(sA_s[:], sA_s_d[:])
            sA_b = const.tile([128, 1], f32)
            nc.sync.dma_start(sA_b[:], sA_b_d[:])
            sB_s = const.tile([128, 3], f32)
            nc.sync.dma_start(sB_s[:], sB_s_d[:])
            sB_b = const.tile([128, 3], f32)
            nc.sync.dma_start(sB_b[:], sB_b_d[:])

            b_qk = const.tile([128, 16, 1], f32)
            nc.sync.dma_start(b_qk[:], b_qk_d.rearrange("(k p) o -> p k o", p=128))
            b_v = const.tile([1, E], f32)
            nc.sync.dma_start(b_v[:], b_v_d[:])
            b_ap = const.tile([128, KT, 1], f32)
            nc.sync.dma_start(b_ap[:], b_ap_d.rearrange("(k p) o -> p k o", p=128))
            b_fc = const.tile([128, 32, 1], f32)
            nc.sync.dma_start(b_fc[:], b_fc_d.rearrange("(k p) o -> p k o", p=128))
            b_pr = const.tile([128, KT, 1], f32)
            nc.sync.dma_start(b_pr[:], b_pr_d.rearrange("(k p) o -> p k o", p=128))

            attnTB = aBp.tile([128, KT, CH], bf16)
            h2T = h2p.tile([128, KT, 2 * CH], bf16)
            x2T = x2p.tile([128, KT, 2 * CH], bf16)

            # window-scoped pools (opened/closed in LIFO order)
            aAp = stack.enter_context(tc.tile_pool(name="aAp", bufs=1))
            attnTA = aAp.tile([128, KT, CH], bf16)
            attnTs = [attnTA, attnTB]

            atsb_ctx = ExitStack()
            atsb = atsb_ctx.enter_context(tc.tile_pool(name="atsb", bufs=4))

            # ---------------- Phase 1: LN1 (chunk-pipelined) ----------------
            ln1_ctx = ExitStack()
            ln1 = ln1_ctx.enter_context(tc.tile_pool(name="ln1", bufs=1))
            ln1_t = ln1.tile([128, KT, T], bf16)

            with tc.tile_pool(name="ln1ps", bufs=1, space="PSUM") as lnps, \
                 tc.tile_pool(name="ln1sb", bufs=1) as lnsb:
                state = {}

                def ln1_stats(ch):
                    xt = lnsb.tile([128, KT, CH], f32, tag="xin", bufs=2,
                                   name=f"l1x{ch}")
                    nc.sync.dma_start(xt[:], xTr[:, :, ch * CH:(ch + 1) * CH])
                    mu_ps = lnps.tile([1, CH], f32, tag="stat", bufs=4,
                                      name=f"l1mu{ch}")
                    ss_ps = lnps.tile([1, CH], f32, tag="stat", bufs=4,
                                      name=f"l1ss{ch}")
                    for kt in range(KT):
                        xbf = lnsb.tile([128, CH], bf16, tag="xbf", bufs=2,
                                        name=f"l1xb{ch}_{kt}")
                        if kt % 2 == 0:
                            nc.scalar.activation(xbf[:], xt[:, kt, :], AF.Copy)
                        else:
                            nc.vector.tensor_copy(xbf[:], xt[:, kt, :])
                        sq = lnsb.tile([128, CH], bf16, tag="sq", bufs=2,
                                       name=f"l1sq{ch}_{kt}")
                        nc.scalar.square(sq[:], xbf[:])
                        nc.tensor.matmul(mu_ps[:], ones_col_bf[:], xbf[:],
                                         start=(kt == 0), stop=(kt == KT - 1))
                        nc.tensor.matmul(ss_ps[:], ones_col_bf[:], sq[:],
                                         start=(kt == 0), stop=(kt == KT - 1))
                    state[ch] = (xt, mu_ps, ss_ps)

                def ln1_norm(ch):
                    xt, mu_ps, ss_ps = state.pop(ch)
                    mu = lnsb.tile([1, CH], f32, tag="row", bufs=7)
                    nc.scalar.activation(mu[:], mu_ps[:], AF.Copy,
                                         scale=1.0 / E)
                    ms = lnsb.tile([1, CH], f32, tag="row", bufs=7)
                    nc.scalar.activation(ms[:], ss_ps[:], AF.Copy,
                                         scale=1.0 / E)
                    msq = lnsb.tile([1, CH], f32, tag="row", bufs=7)
                    nc.scalar.square(msq[:], mu[:])
                    var = lnsb.tile([1, CH], f32, tag="row", bufs=7)
                    nc.vector.tensor_sub(var[:], ms[:], msq[:])
                    sd = lnsb.tile([1, CH], f32, tag="row", bufs=7)
                    nc.scalar.activation(sd[:], var[:], AF.Sqrt,
                                         bias=eps_t[:])
                    rstd = lnsb.tile([1, CH], f32, tag="row", bufs=7)
                    nc.vector.reciprocal_approx_fast(rstd[:], sd[:])
                    nmr = lnsb.tile([1, CH], f32, tag="row", bufs=7)
                    nc.vector.scalar_tensor_tensor(
                        nmr[:], mu[:], -1.0, rstd[:], ALU.mult, ALU.mult)
                    a_bc = lnsb.tile([128, CH], f32, tag="bc", bufs=3)
                    nc.gpsimd.partition_broadcast(a_bc[:], rstd[:])
                    c_bc = lnsb.tile([128, CH], f32, tag="bc", bufs=3)
                    nc.gpsimd.partition_broadcast(c_bc[:], nmr[:])
                    for kt in range(KT):
                        t1 = lnsb.tile([128, CH], f32, tag="t1", bufs=2)
                        nc.vector.tensor_mul(t1[:], xt[:, kt, :], a_bc[:])
                        nc.vector.tensor_add(
                            ln1_t[:, kt, ch * CH:(ch + 1) * CH], t1[:], c_bc[:])

                for ch in range(5):
                    if ch < 4:
                        ln1_stats(ch)
                    if ch >= 1:
                        ln1_norm(ch - 1)

            # ---------------- Phase 2: QKV (minus deferred) ----------------
            qA_ctx = ExitStack()
            qAp = qA_ctx.enter_context(tc.tile_pool(name="qAp", bufs=1))
            qTA = qAp.tile([128, KT, CH], bf16)
            qkv_ctx = ExitStack()
            qkvp = qkv_ctx.enter_context(
                tc.tile_pool(name="qkvp", bufs=1, side="right"))
            qTB = qkvp.tile([128, KT, CH], bf16)
            qTs = [qTA, qTB]
            kT = qkvp.tile([128, KT, T], bf16)
            v_aug = qkvp.tile([128, 16, NH * 65], bf16)
            v4 = v_aug.rearrange("p m (h w) -> p m h w", h=NH)

            with tc.tile_pool(name="wqk", bufs=2) as wqk_pool, \
                 tc.tile_pool(name="qkps", bufs=4, space="PSUM") as qkps:
                for g in range(8):
                    panel = wqk_pool.tile([128, KT, 256], bf16, tag="w")
                    nc.sync.dma_start(panel[:], wT[:, :, g * 256:(g + 1) * 256])
                    for mm in range(2):
                        mt = 2 * g + mm
                        is_q = mt < 8
                        n_chunks = 2 if is_q else 4
                        dt_idx = mt if is_q else mt - 8
                        for nq in range(n_chunks):
                            ps = qkps.tile([128, CH], f32, tag="ps")
                            for kt in range(KT):
                                nc.tensor.matmul(
                                    ps[:], panel[:, kt, mm * 128:(mm + 1) * 128],
                                    ln1_t[:, kt, nq * CH:(nq + 1) * CH],
                                    start=(kt == 0), stop=(kt == KT - 1))
                            dst_ap = (qTs[nq][:, dt_idx, :] if is_q else
                                      kT[:, dt_idx, nq * CH:(nq + 1) * CH])
                            nc.scalar.activation(
                                dst_ap, ps[:],
                                AF.Identity, bias=b_qk[:, mt, 0:1])
                for g in range(2):
                    panel = wqk_pool.tile([128, KT, CH], bf16, tag="wv",
                                          bufs=1)
                    nc.sync.dma_start(
                        panel[:], wT[:, :, 2 * E + g * CH:2 * E + (g + 1) * CH])
                    bv_sb = wqk_pool.tile([128, CH], f32, tag="bvs", bufs=1)
                    nc.gpsimd.partition_broadcast(
                        bv_sb[:], b_v[:, g * CH:(g + 1) * CH])
                    bv_sb3 = bv_sb.rearrange("p (h w) -> p h w", h=8)
                    for mv in range(16):
                        ps = qkps.tile([128, CH], f32, tag="ps")
                        for kt in range(KT):
                            nc.tensor.matmul(
                                ps[:], ln1_t[:, kt, mv * 128:(mv + 1) * 128],
                                panel[:, kt, :],
                                start=(kt == 0), stop=(kt == KT - 1))
                        ps3 = ps.rearrange("p (h w) -> p h w", h=8)
                        nc.vector.tensor_add(
                            v4[:, mv, g * 8:(g + 1) * 8, 0:64], ps3[:], bv_sb3[:])
                for mv in range(16):
                    nc.vector.memset(v4[:, mv, :, 64:65], 1.0)

            # attention PSUM pool: s 3x2 banks + av 2 = 8 banks
            atps_ctx = ExitStack()
            atps = atps_ctx.enter_context(
                tc.tile_pool(name="atps", bufs=1, space="PSUM"))

            # ============ attention machinery ============
            pairs_a = [(0, 1, ("diag", 0)), (2, 3, ("diag", 1)),
                       (8, 9, ("drv", "A", 0)), (10, 11, ("drv", "A", 0))]
            pairs_b = [(4, 5, ("diag", 0)), (6, 7, ("diag", 1)),
                       (0, 1, ("drv", "B", 0)), (2, 3, ("drv", "B", 0)),
                       (8, 9, ("drv", "B", 1)), (10, 11, ("drv", "B", 1)),
                       (12, 13, ("drv", "B", 2)), (14, 15, ("drv", "B", 2))]

            avs = {}
            ess = {}

            def emit_s_exp(work, i):
                h, slot, j, npairs, (t0, t1, mk) = work[i]
                ktf = h // 2
                ro = (h % 2) * 64
                s_ps = atps.tile([128, 2 * CH], f32, tag="s", bufs=3)
                for k, t in enumerate((t0, t1)):
                    nc.tensor.matmul(
                        s_ps[:, k * CH:(k + 1) * CH],
                        kT[ro:ro + 64, ktf, t * 128:(t + 1) * 128],
                        qTs[slot][ro:ro + 64, ktf, :],
                        start=True, stop=True)
                es = atsb.tile([128, 2 * CH], bf16, tag="es", bufs=4)
                if mk[0] == "diag":
                    nc.scalar.activation(es[:], s_ps[:], AF.Exp)
                    nc.vector.tensor_mul(es[:], es[:], pmask[:, mk[1], :])
                else:
                    sc = sA_s if mk[1] == "A" else sB_s
                    bi = sA_b if mk[1] == "A" else sB_b
                    idx = mk[2]
                    nc.scalar.activation(
                        es[:], s_ps[:], AF.Exp,
                        bias=bi[:, idx:idx + 1],
                        scale=sc[:, idx:idx + 1])
                ess[(h, slot, j)] = es

            def emit_pv(work, i):
                h, slot, j, npairs, (t0, t1, mk) = work[i]
                if j == 0:
                    avs[(h, slot)] = atps.tile([65, CH], f32, tag="av",
                                               bufs=2, name=f"av{h}_{slot}")
                out_ps = avs[(h, slot)]
                es = ess.pop((h, slot, j))
                for k, t in enumerate((t0, t1)):
                    nc.tensor.matmul(
                        out_ps[:], v_aug[:, t, h * 65:(h + 1) * 65],
                        es[:, k * CH:(k + 1) * CH],
                        start=(j == 0 and k == 0),
                        stop=(j == npairs - 1 and k == 1))
                if j == npairs - 1:
                    ktf = h // 2
                    ro = (h % 2) * 64
                    den = atsb.tile([1, CH], f32, tag="dn", bufs=2)
                    nc.vector.tensor_copy(den[:], out_ps[64:65, :])
                    rec = atsb.tile([1, CH], f32, tag="rc", bufs=2)
                    nc.vector.reciprocal_approx_fast(rec[:], den[:])
                    bc64 = atsb.tile([64, CH], f32, tag="bcr", bufs=2)
                    nc.gpsimd.partition_broadcast(bc64[:], rec[:])
                    nc.vector.tensor_mul(
                        attnTs[slot][ro:ro + 64, ktf, :], out_ps[0:64, :],
                        bc64[:])

            def attn_stream(slot):
                seq = pairs_a if slot == 0 else pairs_b
                work = []
                for h in range(NH):
                    for j, p in enumerate(seq):
                        work.append((h, slot, j, len(seq), p))
                LEAD = 2
                for i in range(len(work) + LEAD):
                    if i < len(work):
                        emit_s_exp(work, i)
                    if i >= LEAD:
                        emit_pv(work, i - LEAD)
                    yield

            def ap_ln2_stream(slot, mlsb):
                """attnproj + residual -> LN2 for one slot."""
                nq = slot
                for m in range(KT):
                    wpan = mlsb.tile([128, KT, 128], bf16, tag="wap", bufs=2,
                                     name=f"wap{slot}_{m}")
                    nc.sync.dma_start(
                        wpan[:], wapT[:, :, m * 128:(m + 1) * 128])
                    ps = mlps.tile([128, CH], f32, tag="ps", bufs=2,
                                   name=f"ap{slot}_{m}")
                    for kt in range(KT):
                        nc.tensor.matmul(
                            ps[:], wpan[:, kt, :],
                            attnTs[slot][:, kt, :],
                            start=(kt == 0), stop=(kt == KT - 1))
                    xq = mlsb.tile([128, CH], f32, tag="xq", bufs=2,
                                   name=f"xq{slot}_{m}")
                    nc.sync.dma_start(
                        xq[:], xT_d[m * 128:(m + 1) * 128,
                                    nq * CH:(nq + 1) * CH])
                    nc.vector.scalar_tensor_tensor(
                        x2T[:, m, nq * CH:(nq + 1) * CH], ps[:],
                        b_ap[:, m, 0:1], xq[:], ALU.add, ALU.add)
                    yield
                # LN2 (stats borrow partitions 0/32 of a [128, CH] psum tile;
                # x2 is bf16 so the stats matmuls run in bf16 directly)
                stat_t = mlps.tile([128, CH], f32, tag="ps", bufs=2,
                                   name=f"l2s{slot}")
                mu_ps = stat_t[0:1, :]
                ss_ps = stat_t[32:33, :]
                src = x2T[:, :, nq * CH:(nq + 1) * CH]
                for kt in range(KT):
                    sq = mlsb.tile([128, CH], bf16, tag="sq2", bufs=2,
                                   name=f"l2sq{slot}_{kt}")
                    nc.scalar.square(sq[:], src[:, kt, :])
                    nc.tensor.matmul(mu_ps, ones_col_bf[:], src[:, kt, :],
                                     start=(kt == 0), stop=(kt == KT - 1))
                    nc.tensor.matmul(ss_ps, ones_col_bf[:], sq[:],
                                     start=(kt == 0), stop=(kt == KT - 1))
                    if kt % 2 == 1:
                        yield
                mu = mlsb.tile([1, CH], f32, tag="row2", bufs=7)
                nc.scalar.activation(mu[:], mu_ps, AF.Copy, scale=1.0 / E)
                ms = mlsb.tile([1, CH], f32, tag="row2", bufs=7)
                nc.scalar.activation(ms[:], ss_ps, AF.Copy, scale=1.0 / E)
                msq = mlsb.tile([1, CH], f32, tag="row2", bufs=7)
                nc.scalar.square(msq[:], mu[:])
                var = mlsb.tile([1, CH], f32, tag="row2", bufs=7)
                nc.vector.tensor_sub(var[:], ms[:], msq[:])
                sd = mlsb.tile([1, CH], f32, tag="row2", bufs=7)
                nc.scalar.activation(sd[:], var[:], AF.Sqrt, bias=eps_t[:])
                rstd = mlsb.tile([1, CH], f32, tag="row2", bufs=7)
                nc.vector.reciprocal_approx_fast(rstd[:], sd[:])
                nmr = mlsb.tile([1, CH], f32, tag="row2", bufs=7)
                nc.vector.scalar_tensor_tensor(
                    nmr[:], mu[:], -1.0, rstd[:], ALU.mult, ALU.mult)
                a_bc = mlsb.tile([128, CH], f32, tag="bc2", bufs=2)
                nc.gpsimd.partition_broadcast(a_bc[:], rstd[:])
                c_bc = mlsb.tile([128, CH], f32, tag="bc2", bufs=2)
                nc.gpsimd.partition_broadcast(c_bc[:], nmr[:])
                for kt in range(KT):
                    t1 = mlsb.tile([128, CH], f32, tag="t12", bufs=2)
                    nc.vector.tensor_mul(t1[:], src[:, kt, :], a_bc[:])
                    nc.vector.tensor_add(
                        h2T[:, kt, nq * CH:(nq + 1) * CH], t1[:], c_bc[:])
                    if kt % 2 == 1:
                        yield

            def ap_ln2A_w2(w2sb):
                """attnproj-A + LN2-A inside attention-B, borrowing the
                atps s-ring for psum (validated pattern: half-tile groups).
                The LN2 stats run as one unyielding burst so the borrowed
                s-ring slot releases promptly (avoids PE queue deadlock)."""
                for mpair in range(4):
                    wpan = w2sb.tile([128, KT, 256], bf16, tag="wapA",
                                     bufs=2, name=f"wapA{mpair}")
                    nc.sync.dma_start(
                        wpan[:], wapT[:, :, mpair * 256:(mpair + 1) * 256])
                    sp = atps.tile([128, 2 * CH], f32, tag="s", bufs=3,
                                   name=f"apw{mpair}")
                    for half in range(2):
                        m = 2 * mpair + half
                        ps = sp[:, half * CH:(half + 1) * CH]
                        for kt in range(KT):
                            nc.tensor.matmul(
                                ps, wpan[:, kt, half * 128:(half + 1) * 128],
                                attnTs[0][:, kt, :],
                                start=(kt == 0), stop=(kt == KT - 1))
                        xq = w2sb.tile([128, CH], f32, tag="xqA", bufs=2,
                                       name=f"xqA{m}")
                        nc.sync.dma_start(
                            xq[:], xT_d[m * 128:(m + 1) * 128, 0:CH])
                        nc.vector.scalar_tensor_tensor(
                            x2T[:, m, 0:CH], ps, b_ap[:, m, 0:1], xq[:],
                            ALU.add, ALU.add)
                        yield
                st = atps.tile([128, 2 * CH], f32, tag="s", bufs=3,
                               name="l2sA")
                mu_ps = st[0:1, 0:CH]
                ss_ps = st[32:33, 0:CH]
                src0 = x2T[:, :, 0:CH]
                for kt in range(KT):
                    sq = w2sb.tile([128, CH], bf16, tag="sqA", bufs=2,
                                   name=f"l2sqA{kt}")
                    nc.scalar.square(sq[:], src0[:, kt, :])
                    nc.tensor.matmul(mu_ps, ones_col_bf[:], src0[:, kt, :],
                                     start=(kt == 0), stop=(kt == KT - 1))
                    nc.tensor.matmul(ss_ps, ones_col_bf[:], sq[:],
                                     start=(kt == 0), stop=(kt == KT - 1))
                mu = w2sb.tile([1, CH], f32, tag="rowA", bufs=7)
                nc.scalar.activation(mu[:], mu_ps, AF.Copy, scale=1.0 / E)
                ms = w2sb.tile([1, CH], f32, tag="rowA", bufs=7)
                nc.scalar.activation(ms[:], ss_ps, AF.Copy, scale=1.0 / E)
                yield
                msq = w2sb.tile([1, CH], f32, tag="rowA", bufs=7)
                nc.scalar.square(msq[:], mu[:])
                var = w2sb.tile([1, CH], f32, tag="rowA", bufs=7)
                nc.vector.tensor_sub(var[:], ms[:], msq[:])
                sd = w2sb.tile([1, CH], f32, tag="rowA", bufs=7)
                nc.scalar.activation(sd[:], var[:], AF.Sqrt, bias=eps_t[:])
                rstd = w2sb.tile([1, CH], f32, tag="rowA", bufs=7)
                nc.vector.reciprocal_approx_fast(rstd[:], sd[:])
                nmr = w2sb.tile([1, CH], f32, tag="rowA", bufs=7)
                nc.vector.scalar_tensor_tensor(
                    nmr[:], mu[:], -1.0, rstd[:], ALU.mult, ALU.mult)
                a_bc = w2sb.tile([128, CH], f32, tag="bcA", bufs=2)
                nc.gpsimd.partition_broadcast(a_bc[:], rstd[:])
                c_bc = w2sb.tile([128, CH], f32, tag="bcA", bufs=2)
                nc.gpsimd.partition_broadcast(c_bc[:], nmr[:])
                yield
                for kt in range(KT):
                    t1 = w2sb.tile([128, CH], f32, tag="t1A", bufs=2)
                    nc.vector.tensor_mul(t1[:], src0[:, kt, :], a_bc[:])
                    nc.vector.tensor_add(h2T[:, kt, 0:CH], t1[:], c_bc[:])
                    if kt % 2 == 1:
                        yield

            def fc_stream(slot, g_t, mlsb):
                nq = slot
                for mg in range(8):
                    panel = mlsb.tile([128, KT, CH], bf16, tag="wfc", bufs=2,
                                      name=f"fc{slot}_{mg}")
                    nc.sync.dma_start(
                        panel[:], wfcT[:, :, mg * CH:(mg + 1) * CH])
                    for mm in range(4):
                        ps = mlps.tile([128, CH], f32, tag="ps", bufs=2,
                                       name=f"fc{slot}_{mg}_{mm}")
                        for kt in range(KT):
                            nc.tensor.matmul(
                                ps[:], panel[:, kt, mm * 128:(mm + 1) * 128],
                                h2T[:, kt, nq * CH:(nq + 1) * CH],
                                start=(kt == 0), stop=(kt == KT - 1))
                        mt = mg * 4 + mm
                        nc.scalar.activation(
                            g_t[:, mt, nq * CH:(nq + 1) * CH], ps[:],
                            AF.Gelu, bias=b_fc[:, mt, 0:1])
                        yield

            def proj_stream(slot, g_t, prps, prsb):
                nq = slot
                for half in range(2):
                    ms = range(half * 4, (half + 1) * 4)
                    pss = [prps.tile([128, CH], f32, tag="ps", bufs=4,
                                     name=f"pr{slot}_{half}_{m}")
                           for m in ms]
                    for kg in range(8):
                        panel = prsb.tile([128, 4, E], bf16, tag="w", bufs=2,
                                          name=f"prw{slot}_{half}_{kg}")
                        nc.sync.dma_start(
                            panel[:], wprT[:, 4 * kg:4 * kg + 4, :])
                        for kk in range(4):
                            kt = kg * 4 + kk
                            for mi, m in enumerate(ms):
                                nc.tensor.matmul(
                                    pss[mi][:],
                                    panel[:, kk, m * 128:(m + 1) * 128],
                                    g_t[:, kt, nq * CH:(nq + 1) * CH],
                                    start=(kt == 0), stop=(kt == 31),
                                    skip_group_check=True)
                        yield
                    for mi, m in enumerate(ms):
                        ot = prsb.tile([128, CH], f32, tag="ot", bufs=3,
                                       name=f"ot{slot}_{half}_{m}")
                        nc.vector.scalar_tensor_tensor(
                            ot[:], pss[mi][:], b_pr[:, m, 0:1],
                            x2T[:, m, nq * CH:(nq + 1) * CH],
                            ALU.add, ALU.add)
                        nc.sync.dma_start(
                            out_d[m * 128:(m + 1) * 128,
                                  nq * CH:(nq + 1) * CH],
                            ot[:])
                        yield

            def chain(*gens):
                for g in gens:
                    for _ in g:
                        yield

            def interleave(main, filler, ratio):
                budget = 0.0
                for _ in main:
                    budget += ratio
                    while budget >= 1.0 and filler is not None:
                        budget -= 1.0
                        try:
                            next(filler)
                        except StopIteration:
                            filler = None
                if filler is not None:
                    for _ in filler:
                        pass

            # ---- window 1: attention A ----
            for _ in attn_stream(0):
                pass
            qA_ctx.close()
            ln1_ctx.close()

            # ---- window 2: attention B || attnproj-A + LN2-A ----
            w2_ctx = ExitStack()
            w2sb = w2_ctx.enter_context(tc.tile_pool(name="w2sb", bufs=1))
            interleave(attn_stream(1), ap_ln2A_w2(w2sb), 16 / 130.0)
            w2_ctx.close()
            atsb_ctx.close()
            atps_ctx.close()
            qkv_ctx.close()

            # ---- window 3: MLP both slots, proj-A overlapped ----
            gp = stack.enter_context(tc.tile_pool(name="gp", bufs=1, side="right"))
            gT = gp.tile([128, 32, 2 * CH], bf16)
            prsb = stack.enter_context(tc.tile_pool(name="prsb", bufs=1))
            mlps_ctx = ExitStack()
            mlps = mlps_ctx.enter_context(
                tc.tile_pool(name="mlps", bufs=1, space="PSUM"))
            prps_ctx = ExitStack()
            prps = prps_ctx.enter_context(
                tc.tile_pool(name="prps", bufs=1, space="PSUM"))
            mlB_ctx = ExitStack()
            mlsbB = mlB_ctx.enter_context(tc.tile_pool(name="mlsbB", bufs=1))

            interleave(
                chain(ap_ln2_stream(1, mlsbB), fc_stream(1, gT, mlsbB)),
                chain(fc_stream(0, gT, mlsbB),
                      proj_stream(0, gT, prps, prsb)),
                56 / 48.0)
            mlB_ctx.close()

            # ---- window 4: proj-B ----
            for _ in proj_stream(1, gT, prps, prsb):
                pass
            prps_ctx.close()
            mlps_ctx.close()

    nc.compile()
    return nc


def _host_prep(inputs):
    """Build the 8 per-core input maps."""
    x = np.asarray(inputs["x"], np.float32)
    ln1_g = np.asarray(inputs["ln1_g"], np.float32)
    ln1_b = np.asarray(inputs["ln1_b"], np.float32)
    ln2_g = np.asarray(inputs["ln2_g"], np.float32)
    ln2_b = np.asarray(inputs["ln2_b"], np.float32)

    # Fold LN1 gamma/beta into the QKV GEMM, and 1/sqrt(head_dim) into Q.
    w_attn_raw = np.asarray(inputs["w_attn"], np.float32)
    w_attn = (w_attn_raw * ln1_g[:, None]).copy()
    b_attn = (np.asarray(inputs["b_attn"], np.float32)
              + ln1_b @ w_attn_raw).copy()
    w_attn[:, :E] *= 0.125
    b_attn[:E] *= 0.125
    w_attn_bf = np.ascontiguousarray(w_attn.astype(BF))
    b_qk = np.ascontiguousarray(b_attn[:2 * E].reshape(2 * E, 1))
    b_v = np.ascontiguousarray(b_attn[2 * E:].reshape(1, E))

    # Fold LN2 gamma/beta into the FC GEMM.
    w_fc_raw = np.asarray(inputs["w_fc"], np.float32)
    w_fc = w_fc_raw * ln2_g[:, None]
    b_fc = np.asarray(inputs["b_fc"], np.float32) + ln2_b @ w_fc_raw

    w_ap_bf = np.ascontiguousarray(np.asarray(inputs["w_attnproj"], np.float32).astype(BF))
    w_fc_bf = np.ascontiguousarray(w_fc.astype(BF))
    w_pr_bf = np.ascontiguousarray(np.asarray(inputs["w_proj"], np.float32).astype(BF))
    col = lambda v: np.ascontiguousarray(np.asarray(v, np.float32).reshape(-1, 1))
    b_ap = col(inputs["b_attnproj"])
    b_fc = col(b_fc)
    b_pr = col(inputs["b_proj"])

    # static diagonal pair masks (bf16 0/1, applied post-exp):
    # within a 512-chunk, kv tile t allows query col j iff j >= t*128 + p.
    j = np.arange(CH)[None, :]
    p = np.arange(128)[:, None]
    m01 = [np.where(j >= t * 128 + p, 1.0, 0.0).astype(np.float32)
           for t in range(4)]
    pm = np.stack([np.concatenate([m01[0], m01[1]], axis=1),
                   np.concatenate([m01[2], m01[3]], axis=1)])
    pm_bf = np.ascontiguousarray(pm.astype(BF))

    ON = (1.0, 0.0)
    OFF = (0.0, NEG)
    in_maps = []
    perms = []
    for core in range(8):
        b = core // 2
        z = core % 2
        blocks = [0, 3, 1, 2] if z == 0 else [1, 2, 0, 3]
        perms.append(blocks)
        cols = np.concatenate([np.arange(c * CH, (c + 1) * CH) for c in blocks])
        xT = np.ascontiguousarray(x[b].T[:, cols])
        # slot A: driven block = O1 (perm pos 2); allowed iff block(O1) < block(A)
        sa = ON if blocks[2] < blocks[0] else OFF
        # slot B: driven = A, O1, O2 (perm pos 0, 2, 3) vs chunk B
        sbs = [ON if blocks[i] < blocks[1] else OFF for i in (0, 2, 3)]
        f = np.float32
        in_maps.append({
            "xT": xT,
            "w_attn": w_attn_bf, "b_qk": b_qk, "b_v": b_v,
            "w_ap": w_ap_bf, "b_ap": b_ap,
            "w_fc": w_fc_bf, "b_fc": b_fc, "w_proj": w_pr_bf, "b_proj": b_pr,
            "pmask": pm_bf,
            "sA_scale": np.full((128, 1), sa[0], f),
            "sA_bias": np.full((128, 1), sa[1], f),
            "sB_scale": np.ascontiguousarray(
                np.tile(np.array([[s for s, _ in sbs]], f), (128, 1))),
            "sB_bias": np.ascontiguousarray(
                np.tile(np.array([[bb for _, bb in sbs]], f), (128, 1))),
        })
    return in_maps, perms


def _run(inputs, trace=False):
    from concourse.bass_utils import run_bass_kernel_spmd

    if "nc" not in _CACHE:
        _CACHE["nc"] = _build_program()
    nc = _CACHE["nc"]
    in_maps, perms = _host_prep(inputs)
    res = run_bass_kernel_spmd(nc, in_maps, list(range(8)), trace=trace)
    x = np.asarray(inputs["x"], np.float32)
    out = np.empty_like(x)
    for core in range(8):
        b = core // 2
        blocks = perms[core]
        oT = res.results[core]["outT"]
        cA, cB = blocks[0], blocks[1]
        out[b, cA * CH:(cA + 1) * CH, :] = oT[:, 0:CH].T
        out[b, cB * CH:(cB + 1) * CH, :] = oT[:, CH:2 * CH].T
    return out, res


def kernel(**inputs) -> np.ndarray:
    out, _ = _run(inputs, trace=False)
    return out
